# revision 19
# baseline (speedup 1.0000x reference)
"""Trainium2 Bass kernel for a single attention head with query-axis softmax.

Reference semantics (per batch b):
    k = x @ Wk; q = x @ Wq; v = x @ Wv                 # [T, H]
    wei = (q @ k^T) * E**-0.5                          # [T(query), T(key)]
    wei = where(tril, wei, -inf)                       # causal: keep s <= t
    p = softmax(wei, axis=0 over query t)              # NOTE: query axis!
    out = p @ v                                        # [T, H]

Because the softmax normalizes over the query axis t (per key column s),
out[t,h] = sum_s E[t,s] * v[s,h] / d[s] with E[t,s] = exp(wei[t,s])
(zero for s > t) and d[s] = sum_t E[t,s].  The kernel computes E^T tiles
([s on partitions, t free]) so d is a free-axis row sum (fused into the
exp instruction via accum_out), scales v rows by 1/d, and accumulates
out^T on PE.  Cross-partition layout fixes (v^T -> v, out^T -> out) ride
the DMA transpose crossbar (bf16).

Sharding: batch dim (8) across the 8 NeuronCores, weights replicated.
The host passes x pre-transposed per batch (xT[b] = x[b].T) in bf16 so
no on-device transpose of the big activation tensor is needed; matmul
operands are bf16 with fp32 PSUM accumulation.
"""

import numpy as np
import ml_dtypes

import concourse.bass as bass
import concourse.tile as tile
from concourse import bacc, mybir
from concourse import bass_utils

B, T, E, H = 8, 2048, 1024, 64
P = 128                       # partitions
CB = 512                      # column block (t) width
NE = E // P                   # 8 contraction chunks for projections
NJ = T // CB                  # 4 column blocks
NI = T // P                   # 16 s-chunks
SCALE = float(E) ** -0.5      # note: embed**-0.5, not head_size**-0.5
MASK_NEG = -1.0e30
F32 = mybir.dt.float32
F32R = mybir.dt.float32r
BF16 = mybir.dt.bfloat16
X = mybir.AxisListType.X
EXP = mybir.ActivationFunctionType.Exp


def _emit(tc, xT_d, wq_d, wkv_d, masks_d, out_d):
    nc = tc.nc
    from contextlib import ExitStack

    with ExitStack() as ctx:
        singles = ctx.enter_context(tc.tile_pool(name="singles", bufs=1))
        xpool = ctx.enter_context(tc.tile_pool(name="x", bufs=1))
        epool = ctx.enter_context(tc.tile_pool(name="erow", bufs=6))
        dpool = ctx.enter_context(tc.tile_pool(name="dsmall", bufs=8))
        vpool = ctx.enter_context(tc.tile_pool(name="vrow", bufs=6))
        opool = ctx.enter_context(tc.tile_pool(name="onat", bufs=8))
        ps = ctx.enter_context(tc.tile_pool(name="ps", bufs=2, space="PSUM"))
        pproj_pool = ctx.enter_context(
            tc.tile_pool(name="pproj", bufs=1, space="PSUM")
        )
        pout = ctx.enter_context(tc.tile_pool(name="pout", bufs=1, space="PSUM"))

        # x^T viewed as [partition, e-chunk, t]; packets of one dma_start
        # spread across all 16 SDMA engines, so few big DMAs beat many
        # small ones (issue is ~0.6us each on the sequencer)
        xT_v = xT_d.rearrange("(ne p) t -> p ne t", p=P)
        # j=3 column first so the first projection chain starts early
        xts3 = xpool.tile([P, NE, CB], BF16, tag="xt3", name="xts3")
        nc.sync.dma_start(out=xts3[:], in_=xT_v[:, :, 3 * CB :])
        # weights host-packed: wq [128, 8*64], wkv [128, 8*128]
        wq_sb = singles.tile([P, NE * H], BF16)
        nc.scalar.dma_start(out=wq_sb[:], in_=wq_d[:])
        wkv_sb = singles.tile([P, NE * 2 * H], BF16)
        nc.scalar.dma_start(out=wkv_sb[:], in_=wkv_d[:])
        # 4 additive causal triangle masks [128, 4*128] + f32r identity
        masks_sb = singles.tile([P, 4 * P], F32)
        nc.scalar.dma_start(out=masks_sb[:], in_=masks_d[:, 0 : 4 * P])
        identr = singles.tile([P, P], F32R)
        nc.scalar.dma_start(
            out=identr[:], in_=masks_d[:, 4 * P : 5 * P].bitcast(F32R)
        )
        # rest of x^T (j = 0..2 columns)
        xtsr = xpool.tile([P, NE, 3 * CB], BF16, tag="xtr", name="xtsr")
        nc.sync.dma_start(out=xtsr[:], in_=xT_v[:, :, 0 : 3 * CB])

        def x_rhs(e, j):
            if j == 3:
                return xts3[:, e, :]
            return xtsr[:, e, j * CB : (j + 1) * CB]

        # persistent activations
        q_sb = singles.tile([H, T], BF16)    # q^T
        kT_sb = singles.tile([H, T], BF16)   # k^T
        vT_sb = singles.tile([P, T], F32R)   # v^T lives in rows 64:128
        outT_sb = singles.tile([P, T // 2], F32R)  # rows 0:64 jj even, 64:128 odd

        # out^T accumulators packed 2 per bank: jj even rows 0:64, odd 64:128.
        # Accumulation groups on disjoint partition ranges of one bank are
        # fine on HW (per-element has_written); skip the sim's coarse check.
        pout_tiles = [
            pout.tile([P, CB], F32, tag=f"pt{a}", name=f"pt{a}") for a in range(2)
        ]

        def pout_slice(jj, c0, c1):
            rb = H * (jj % 2)
            return pout_tiles[jj // 2][rb : rb + H, c0:c1]

        # deferred AV emission (lag one row behind S so PE never waits on
        # the d / v' chain): each entry = (r, d0, erow, vi, j_of_row)
        pending_av = []

        def _av_one(rj, d0, erow, vi, jj):
            c = (jj - rj[1]) * CB
            lo = d0 if jj == rj[1] else 0
            nc.tensor.matmul(
                pout_slice(jj, lo, CB),
                lhsT=vi[:],
                rhs=erow[:, c + lo : c + CB],
                start=(jj == rj[1] and rj[0] == 0),
                stop=(rj[1] == 0 and rj[0] == 3),
                skip_group_check=True,
            )

        def flush_av(final):
            if final:
                # group by output bank so bank A closes early and its
                # finale transposes overlap bank B's last matmuls
                rows = list(pending_av)
                pending_av.clear()
                for jj in range(NJ):
                    for rj, d0, erow, vi in rows:
                        if jj >= rj[1]:
                            _av_one(rj, d0, erow, vi, jj)
                return
            while pending_av:
                rj, d0, erow, vi = pending_av.pop(0)
                for jj in range(rj[1], NJ):
                    _av_one(rj, d0, erow, vi, jj)

        # projection matmul emission is spread through the PREVIOUS step's
        # rows so the PE instruction stream stays dense (HAM stays warm)
        def proj_thunks(j):
            pproj = pproj_pool.tile([P, 2 * CB], F32, tag="pp", name="pproj")
            thunks = []
            for e in range(NE):
                thunks.append(
                    lambda e=e: nc.tensor.matmul(
                        pproj[0:H, 0:CB],
                        lhsT=wq_sb[:, e * H : (e + 1) * H],
                        rhs=x_rhs(e, j),
                        start=(e == 0),
                        stop=(e == NE - 1),
                    )
                )
            for e in range(NE):
                thunks.append(
                    lambda e=e: nc.tensor.matmul(
                        pproj[:, CB : 2 * CB],
                        lhsT=wkv_sb[:, e * 2 * H : (e + 1) * 2 * H],
                        rhs=x_rhs(e, j),
                        start=(e == 0),
                        stop=(e == NE - 1),
                    )
                )
            return pproj, thunks

        def proj_casts(j, pproj):
            t0 = j * CB
            nc.vector.tensor_copy(q_sb[:, t0 : t0 + CB], pproj[0:H, 0:CB])
            nc.vector.tensor_copy(kT_sb[:, t0 : t0 + CB], pproj[0:H, CB : 2 * CB])
            nc.vector.tensor_copy(vT_sb[H:P, t0 : t0 + CB], pproj[H:P, CB : 2 * CB])

        # --- main pipeline: column blocks in descending order --------------
        next_proj = []  # pending matmul thunks for step j-1's projections

        def drip_proj(k):
            for _ in range(min(k, len(next_proj))):
                next_proj.pop(0)()

        pproj, thunks = proj_thunks(3)
        for t in thunks:
            t()
        proj_casts(3, pproj)

        for j in reversed(range(NJ)):
            if j > 0:
                pproj_next, next_proj = proj_thunks(j - 1)

            # rows i = 4j .. 4j+3 of E^T are now computable in full
            for r in range(4):
                i = 4 * j + r
                s0 = i * P
                d0 = r * P  # first unmasked column of the diagonal block
                nblk = NJ - j
                erow = epool.tile([P, T], BF16)
                dparts = dpool.tile([P, 2], F32, tag="dparts")
                npair = (nblk + 1) // 2
                for pair in range(npair):
                    jj0 = j + 2 * pair
                    w = CB * min(2, NJ - jj0)  # 512 or 1024
                    pst = ps.tile([P, 2 * CB], F32, tag="ps")
                    for u in range(w // CB):
                        jj = jj0 + u
                        lo = d0 if jj == j else 0
                        nc.tensor.matmul(
                            pst[:, u * CB + lo : (u + 1) * CB],
                            lhsT=kT_sb[:, s0 : s0 + P],
                            rhs=q_sb[:, jj * CB + lo : (jj + 1) * CB],
                            start=True,
                            stop=True,
                        )
                    drip_proj(3)
                    lo = d0 if pair == 0 else 0
                    if pair == 0:
                        # additive -1e30 triangle on the partial subblock
                        nc.vector.tensor_add(
                            pst[:, lo : lo + P],
                            pst[:, lo : lo + P],
                            masks_sb[:, r * P : (r + 1) * P],
                        )
                    c = 2 * CB * pair
                    nc.scalar.activation(
                        out=erow[:, c + lo : c + w],
                        in_=pst[:, lo:w],
                        func=EXP,
                        scale=SCALE,
                        accum_out=dparts[:, pair : pair + 1],
                    )

                # d = sum of block sums; v' = v / d (v arrives via DMA
                # transpose of the v^T slice, then a per-partition scale)
                dinv = dpool.tile([P, 1], F32, tag="dinv")
                if npair > 1:
                    dsum = dpool.tile([P, 1], F32, tag="dsum")
                    nc.vector.reduce_sum(dsum[:], dparts[:, 0:npair], axis=X)
                    nc.vector.reciprocal(dinv[:], dsum[:])
                else:
                    nc.vector.reciprocal(dinv[:], dparts[:, 0:1])

                pvt = ps.tile([P, 2 * CB], F32R, tag="ps", name="pvt")
                nc.tensor.transpose(
                    pvt[:, 0:H], vT_sb[H:P, s0 : s0 + P], identr[H:P, H:P]
                )
                vi = vpool.tile([P, H], BF16, tag="vi", name="vi")
                nc.vector.tensor_scalar_mul(vi[:], pvt[:, 0:H], dinv[:])

                if j > 0:
                    flush_av(False)  # previous row's AV matmuls go here
                drip_proj(2)
                pending_av.append(((r, j), d0, erow, vi))

            # drain remaining next-step projection matmuls, then its casts
            drip_proj(len(next_proj))
            if j > 0:
                proj_casts(j - 1, pproj_next)

        flush_av(True)

        # --- finale: out^T -> out natural (PE transpose), one store DMA ----
        for a in range(2):
            nc.vector.tensor_copy(
                outT_sb[:, a * CB : (a + 1) * CB], pout_tiles[a][:]
            )
        onf = singles.tile([P, NI, H], F32)
        for c in range(NI):
            jj = c // 4
            rb = H * (jj % 2)
            col = (jj // 2) * CB + (c % 4) * P
            pso = ps.tile([P, 2 * CB], F32R, tag="ps", name="pso")
            nc.tensor.transpose(
                pso[:, 0:H],
                outT_sb[rb : rb + H, col : col + P],
                identr[rb : rb + H, rb : rb + H],
            )
            nc.vector.tensor_copy(onf[:, c, :], pso[:, 0:H])
        nc.sync.dma_start(
            out=out_d.rearrange("(c p) h -> p c h", p=P), in_=onf[:]
        )


def _enable_ldw_opt():
    """Flip walrus's --enable-ldw-opt to true for our compile: consecutive
    matmuls reusing the same stationary operand then skip the reload."""
    import concourse.bass_utils as bu

    if getattr(bu, "_ldw_opt_patched", False):
        return
    orig = bu.run_command

    def run_command_ldw(cmd, *a, **kw):
        if isinstance(cmd, list):
            cmd = [
                "--enable-ldw-opt=true" if c == "--enable-ldw-opt=false" else c
                for c in cmd
            ]
        return orig(cmd, *a, **kw)

    bu.run_command = run_command_ldw
    bu._ldw_opt_patched = True


def _build_program():
    nc = bacc.Bacc("TRN2", target_bir_lowering=False, debug=False, num_devices=B)
    xT_d = nc.dram_tensor("xT", [E, T], BF16, kind="ExternalInput").ap()
    wq_d = nc.dram_tensor("wq", [P, NE * H], BF16, kind="ExternalInput").ap()
    wkv_d = nc.dram_tensor("wkv", [P, NE * 2 * H], BF16, kind="ExternalInput").ap()
    masks_d = nc.dram_tensor("masks", [P, 5 * P], F32, kind="ExternalInput").ap()
    out_d = nc.dram_tensor("out", [T, H], F32, kind="ExternalOutput").ap()
    with tile.TileContext(nc) as tc:
        _emit(tc, xT_d, wq_d, wkv_d, masks_d, out_d)
    nc.compile()
    return nc


def _host_masks():
    """[128, 5*128]: triangle mask r at cols [128r, ..); identity at 4*128."""
    m = np.full((P, 5 * P), MASK_NEG, dtype=np.float32)
    p = np.arange(P)[:, None]
    f = np.arange(P)[None, :]
    for r in range(4):
        m[:, r * P : (r + 1) * P][f >= p] = 0.0
    m[:, 4 * P : 5 * P] = np.eye(P, dtype=np.float32)
    return m


def _host_inputs(x, Wk, Wq, Wv):
    bf = ml_dtypes.bfloat16
    x = np.asarray(x, dtype=np.float32)
    xT = np.ascontiguousarray(np.transpose(x, (0, 2, 1))).astype(bf)  # [B, E, T]

    def pack_w(*ws):
        # [E, h_tot] (concat) -> [128, NE * h_tot]: chunk e at cols e*h_tot
        w = np.concatenate([np.asarray(a, np.float32) for a in ws], axis=1)
        h = w.shape[1]
        return np.ascontiguousarray(
            w.reshape(NE, P, h).transpose(1, 0, 2).reshape(P, NE * h)
        ).astype(bf)

    wq = pack_w(Wq)
    wkv = pack_w(Wk, Wv)
    masks = _host_masks()
    return [
        {"xT": xT[b], "wq": wq, "wkv": wkv, "masks": masks} for b in range(B)
    ]


def _ensure_axon_ntff_hook():
    """The agent image's antenv lacks axon_hooks; synthesize it so
    run_bass_kernel_spmd's trace path can find the NTFF profile hook."""
    import sys
    import types

    if "antenv.axon_hooks" in sys.modules:
        return
    try:
        import antenv

        mod = types.ModuleType("antenv.axon_hooks")
        mod._hook = None

        def set_axon_ntff_profile_hook(h):
            mod._hook = h

        def get_axon_ntff_profile_hook():
            return mod._hook

        mod.set_axon_ntff_profile_hook = set_axon_ntff_profile_hook
        mod.get_axon_ntff_profile_hook = get_axon_ntff_profile_hook
        sys.modules["antenv.axon_hooks"] = mod
        antenv.axon_hooks = mod

        from trn_agent_boot.trn_boot import _ntff_profile_via_ctypes

        hook = _ntff_profile_via_ctypes("/opt/axon/libaxon_pjrt.so")
        if hook is not None:
            mod._hook = hook
    except Exception as e:  # degrade to untraced run
        print(f"NTFF hook setup failed ({e}); tracing will be skipped")


def kernel(x, Wk, Wq, Wv, _trace=False, _trace_kwargs=None):
    if _trace:
        _ensure_axon_ntff_hook()
    in_maps = _host_inputs(x, Wk, Wq, Wv)
    nc = _build_program()
    res = bass_utils.run_bass_kernel_spmd(
        nc, in_maps, list(range(B)), trace=_trace, **(_trace_kwargs or {})
    )
    out = np.stack([res.results[b]["out"] for b in range(B)], axis=0)
    if _trace:
        kernel.last_results = res
    return out.astype(np.float32)
